# revision 12
# baseline (speedup 1.0000x reference)
"""Trainium2 Bass kernel for quantized Linear: out = x @ (w_int8 * scaler[:,None]).T

Problem (hardcoded): x [2, 2048, 4096] f32, weight [4096, 4096] int32 (int8-range
values), weight_scaler [4096] f32 -> out [2, 2048, 4096] f32.

Strategy: 4x2 shard over 8 NeuronCores — tokens (B*S = 4096) split 4 ways,
out_features split 2 ways. Per core the device kernel is a bf16 GEMM running
the PE at its streaming roofline (~216 ns per 128x128x512 matmul):
  - x^T shard [128 p, 32 k, 1024 t] bf16 (host-cast): SBUF-resident, loaded in
    per-k-tile chunks so the first matmul starts as soon as chunk 0 lands.
  - w^T shard [4 ob, 128 p, 32 k, 512 o] int8 on the wire (values 0..126 are
    int8-exact; halves weight DMA vs bf16 — the load ring only sustains
    ~260 GB/s under 8-core contention and bf16 weights made block 1's
    prefetch miss its deadline). Cast to bf16 on DVE per k-tile; o-block 0
    and 1 stream during block 0's compute, blocks 2-3 prefetch one block
    ahead.
  - matmul: lhsT = x^T tile [128 i, 128 t] (stationary), rhs = w^T tile
    [128 i, 512 o] (moving), fp32 accumulation over 32 k-tiles per PSUM bank.
    Block 0 runs k-outer (tolerates streaming arrival); blocks 1-3 run
    m-outer/k-inner so each token-tile's eviction pipelines under compute.
  - eviction: DVE tensor_tensor multiply against a partition-broadcast
    scaler tile (fp32 dequant straight out of PSUM), output DMA on the
    scalar queue.
  - ~29 junk matmuls on a memset tile bridge the PE from preamble-unblock
    (~7us) to first-data (~10us) so the HAM clock-gate opens (1.2 -> 2.4 GHz)
    before the real stream begins and never re-throttles.
The einsum contraction is not sharded, so no collectives are needed.
"""

import numpy as np
import ml_dtypes

# ---- problem constants (hardcoded per contract) ----
B, S, D_IN, D_OUT = 2, 2048, 4096, 4096
T_FULL = B * S  # 4096 tokens
R_SHARDS, C_SHARDS = 4, 2  # token shards x out_feature shards = 8 cores
T_CORE = T_FULL // R_SHARDS  # 1024 tokens per core
O_CORE = D_OUT // C_SHARDS  # 2048 out features per core

P = 128
KT = D_IN // P  # 32 contraction k-tiles
N = 512  # matmul moving free dim / PSUM bank width
OB = O_CORE // N  # 4 o-blocks per core
MT = T_CORE // P  # 8 token subtiles (PSUM banks)
N_WARM = 29  # HAM warmup matmuls
WC1 = 4  # k-tiles per interleaved block-1 weight piece

_CACHE = {}


def _build_bass():
    import concourse.mybir as mybir
    import concourse.tile as tile
    from concourse import bacc
    from contextlib import ExitStack

    nc = bacc.Bacc()
    xT = nc.dram_tensor("xT", [P, KT, T_CORE], mybir.dt.bfloat16, kind="ExternalInput")
    wT = nc.dram_tensor("wT", [OB, P, KT, N], mybir.dt.int8, kind="ExternalInput")
    sc = nc.dram_tensor("sc", [1, O_CORE], mybir.dt.float32, kind="ExternalInput")
    out = nc.dram_tensor("out", [T_CORE, O_CORE], mybir.dt.float32, kind="ExternalOutput")

    with ExitStack() as ctx:
        tc = ctx.enter_context(tile.TileContext(nc))
        const = ctx.enter_context(tc.tile_pool(name="const", bufs=1))
        xres = ctx.enter_context(tc.tile_pool(name="xres", bufs=1))
        wstg = ctx.enter_context(tc.tile_pool(name="wstg", bufs=2))
        wres = ctx.enter_context(tc.tile_pool(name="wres", bufs=2))
        outp = ctx.enter_context(tc.tile_pool(name="outp", bufs=6))
        psum = ctx.enter_context(tc.tile_pool(name="psum", bufs=8, space="PSUM"))

        # scaler broadcast [128, o_core]: needed by the first eviction
        # (~17us), not by the matmul stream — off the critical path.
        scb = const.tile([P, O_CORE], mybir.dt.float32, name="scb")
        nc.gpsimd.dma_start(out=scb[:], in_=sc[:].to_broadcast([P, O_CORE]))

        # PE warmup: junk matmuls during the DMA head so the HAM clock-gate
        # opens before the real stream begins. They write into ps0[0]; the
        # first real matmul there has start=True which clears the bank.
        wm = const.tile([P, P], mybir.dt.bfloat16, name="wm")
        nc.vector.memset(wm[:], 1.0)
        ps0 = [psum.tile([P, N], mybir.dt.float32, name="ps") for _ in range(MT)]
        for _ in range(N_WARM):
            nc.tensor.matmul(ps0[0][:, :P], lhsT=wm[:], rhs=wm[:], start=True, stop=True)

        x_sb = xres.tile([P, KT, T_CORE], mybir.dt.bfloat16)
        w_i = [wstg.tile([P, KT, N], mybir.dt.int8, name="wi") for _ in range(2)]
        w_bf = [wres.tile([P, KT, N], mybir.dt.bfloat16, name="w") for _ in range(2)]

        def cast_w(buf, k0, k1):
            for k in range(k0, k1):
                nc.vector.tensor_copy(w_bf[buf][:, k, :], w_i[buf][:, k, :])

        # Block-0 chunks (x + w) interleaved per k-tile, with block-1's
        # weights woven in as 256KB pieces: a single prefetch queued behind
        # all of block 0's loads misses its deadline under HBM contention
        # and stalls the PE at the block boundary (+HAM re-throttle).
        for k in range(KT):
            nc.sync.dma_start(x_sb[:, k, :], xT[:, k, :])
            nc.sync.dma_start(w_i[0][:, k, :], wT[0, :, k, :])
            cast_w(0, k, k + 1)
            if k % WC1 == WC1 - 1:
                c = k // WC1
                nc.sync.dma_start(
                    w_i[1][:, c * WC1 : (c + 1) * WC1, :],
                    wT[1, :, c * WC1 : (c + 1) * WC1, :],
                )
                cast_w(1, c * WC1, (c + 1) * WC1)

        def evict(ps_t, m, b):
            ot = outp.tile([P, N], mybir.dt.float32)
            nc.vector.tensor_tensor(
                ot[:], ps_t[:], scb[:, b * N : (b + 1) * N], mybir.AluOpType.mult
            )
            nc.scalar.dma_start(out[m * P : (m + 1) * P, b * N : (b + 1) * N], ot[:])

        # ---- block 0: k-outer (tolerates streaming w/x arrival) ----
        for k in range(KT):
            for m in range(MT):
                nc.tensor.matmul(
                    ps0[m][:],
                    lhsT=x_sb[:, k, m * P : (m + 1) * P],
                    rhs=w_bf[0][:, k, :],
                    start=(k == 0),
                    stop=(k == KT - 1),
                )
        for m in range(MT):
            evict(ps0[m], m, 0)

        # ---- blocks 1..3: m-outer / k-inner with pipelined evicts ----
        for b in range(1, OB):
            if b > 1:
                # prefetch + cast during the previous block's compute
                buf = b % 2
                w_i[buf] = wstg.tile([P, KT, N], mybir.dt.int8, name="wi")
                nc.sync.dma_start(w_i[buf][:], wT[b])
                w_bf[buf] = wres.tile([P, KT, N], mybir.dt.bfloat16, name="w")
                cast_w(buf, 0, KT)
            w_sb = w_bf[b % 2]
            for m in range(MT):
                ps = psum.tile([P, N], mybir.dt.float32, name="ps")
                for k in range(KT):
                    nc.tensor.matmul(
                        ps[:],
                        lhsT=x_sb[:, k, m * P : (m + 1) * P],
                        rhs=w_sb[:, k, :],
                        start=(k == 0),
                        stop=(k == KT - 1),
                    )
                evict(ps, m, b)
    nc.finalize()
    return nc


def _shard_inputs(x, weight, weight_scaler):
    """Host-side layout prep + sharding. Returns per-core input maps."""
    x = np.asarray(x, dtype=np.float32).reshape(T_FULL, D_IN)
    weight = np.asarray(weight, dtype=np.int32)
    weight_scaler = np.asarray(weight_scaler, dtype=np.float32)

    xT = np.ascontiguousarray(x.T)  # [i, t]
    wT = np.ascontiguousarray(weight.T)  # [i, o]

    in_maps = []
    for core in range(8):
        tr, oc = divmod(core, C_SHARDS)
        xs = xT[:, tr * T_CORE : (tr + 1) * T_CORE]  # [4096, 1024]
        # -> [p=128, k=32, t] (i = k*128 + p)
        xs = np.ascontiguousarray(xs.reshape(KT, P, T_CORE).transpose(1, 0, 2))
        ws = wT[:, oc * O_CORE : (oc + 1) * O_CORE]  # [4096, 2048]
        # -> [ob=4, p=128, k=32, 512]
        ws = np.ascontiguousarray(ws.reshape(KT, P, OB, N).transpose(2, 1, 0, 3))
        scs = np.ascontiguousarray(
            weight_scaler[oc * O_CORE : (oc + 1) * O_CORE].reshape(1, O_CORE)
        )
        in_maps.append(
            {
                "xT": xs.astype(ml_dtypes.bfloat16),
                "wT": ws.astype(np.int8),
                "sc": scs,
            }
        )
    return in_maps


def kernel(x, weight, weight_scaler):
    from concourse.bass_utils import run_bass_kernel_spmd

    if "nc" not in _CACHE:
        _CACHE["nc"] = _build_bass()
    nc = _CACHE["nc"]

    in_maps = _shard_inputs(x, weight, weight_scaler)
    res = run_bass_kernel_spmd(nc, in_maps, list(range(8))).results

    out = np.empty((T_FULL, D_OUT), np.float32)
    for core in range(8):
        tr, oc = divmod(core, C_SHARDS)
        out[tr * T_CORE : (tr + 1) * T_CORE, oc * O_CORE : (oc + 1) * O_CORE] = res[
            core
        ]["out"]
    return out.reshape(B, S, D_OUT)
